# revision 50
# baseline (speedup 1.0000x reference)
"""Trainium2 Bass kernel for nn_NoSoftmaxGPT2Model (4-layer GPT2, no softmax).

Strategy: the missing softmax makes attention linear, so (Q K^T) V is
reassociated to Q (K^T V) -- K^T V is only [64, 64] per head. This kills the
S x S attention entirely and makes every op except that contraction
token-local. We shard the 2048-token sequence across 8 NeuronCores (256
tokens each), replicate the weights, and per layer AllReduce only the tiny
[12, 64, 64] K^T V partial sums (bf16, 96 KB).

On-chip layout: activations live in SBUF transposed, [feature_part, token_free]
(T-layout). LayerNorm stats (per-token sums over features = partition
reduction) are computed with ones-vector matmuls on the PE; the stat matmuls
are interleaved into the producing GEMM loops (Wo / MLP2 run m-outer) so they
hide under PE work, and the scalar chain uses reciprocal_approx_fast. Stats
are broadcast back with a rank-1 ones matmul. LN gains are folded into the
following weight matrices on the host; all bias tables are packed host-side
into one contiguous [128, x] tensor (no strided descriptor storms).

Host-side preprocess folds LN gains into weights, packs bias tables
contiguously, pre-transposes the input slice to T-layout (and adds wpe), and
transposes the T-layout output back. kernel(**inputs) takes the full
unsharded inputs and returns the full [1, 2048, 768] output.
"""

import os
from contextlib import ExitStack

import numpy as np
import ml_dtypes

import jax
from jax.sharding import Mesh, PartitionSpec, NamedSharding

import concourse.bass as bass
import concourse.bacc as bacc
import concourse.mybir as mybir
import concourse.tile as tile
from concourse.tile import add_dep_helper
from concourse import bass2jax

from jax.experimental.shard_map import shard_map

N_CORES = 8
L, S, E, H, FF = 4, 2048, 768, 12, 3072
DH = E // H  # 64
T = S // N_CORES  # 256 tokens per core
KT = E // 128  # 6 feature tiles
FT = FF // 128  # 24 ff tiles
EPS = 1e-5

F32 = mybir.dt.float32
AF = mybir.ActivationFunctionType
AO = mybir.AluOpType

# "f32" | "bf16"
COMPUTE = os.environ.get("KERNEL_COMPUTE", "bf16")
AR_F32 = os.environ.get("KERNEL_AR_F32") == "1"  # AllReduce payload in f32
NO_ILV = os.environ.get("KERNEL_NO_ILV") == "1"  # un-interleave stat groups

# packed f32 table column offsets: bq | bo | b2 | b1 | lnfg | lnfb
TAB_BQ = 0
TAB_BO = TAB_BQ + L * KT
TAB_B2 = TAB_BO + L * KT
TAB_B1 = TAB_B2 + L * KT
TAB_LNFG = TAB_B1 + L * FT
TAB_LNFB = TAB_LNFG + KT
TABC = TAB_LNFB + KT


def _dtw():
    return mybir.dt.bfloat16 if COMPUTE == "bf16" else mybir.dt.float32


def build_model(reps=1, n_layers=L, collective=True):
    dtw = _dtw()
    nc = bacc.Bacc(
        "TRN2", target_bir_lowering=False, debug=False, num_devices=N_CORES
    )

    xt_d = nc.dram_tensor("xt", [E, T], F32, kind="ExternalInput").ap()
    wq_d = nc.dram_tensor("wq", [L, E, E], dtw, kind="ExternalInput").ap()
    wkv_d = nc.dram_tensor("wkv", [L, E, 2 * E], dtw, kind="ExternalInput").ap()
    wo_d = nc.dram_tensor("wo", [L, E, E], dtw, kind="ExternalInput").ap()
    w1_d = nc.dram_tensor("w1", [L, E, FF], dtw, kind="ExternalInput").ap()
    w2_d = nc.dram_tensor("w2", [L, FF, E], dtw, kind="ExternalInput").ap()
    tabs_d = nc.dram_tensor("tabs", [128, TABC], F32, kind="ExternalInput").ap()
    bkv_d = nc.dram_tensor("bkv", [L, 2 * E], dtw, kind="ExternalInput").ap()
    out_d = nc.dram_tensor("out", [E, T], F32, kind="ExternalOutput").ap()

    with tile.TileContext(nc) as tc, ExitStack() as ctx:
        const = ctx.enter_context(tc.tile_pool(name="const", bufs=1))
        wpool = ctx.enter_context(tc.tile_pool(name="wpool", bufs=1))
        apool = ctx.enter_context(tc.tile_pool(name="apool", bufs=1))
        ps = ctx.enter_context(tc.tile_pool(name="ps", bufs=1, space="PSUM"))
        pstat = ctx.enter_context(tc.tile_pool(name="pstat", bufs=1, space="PSUM"))
        dram = ctx.enter_context(tc.tile_pool(name="dram", bufs=1, space="DRAM"))

        PP = 6  # ps pool rotation depth (pstat takes the other 2 banks)

        _prev_dma = [None]

        def sdma(dst, src):
            """sync-queue DMA with forced emission-order enqueue (prevents
            scheduler-reordered slot-wait deadlocks in the shared FIFO)."""
            inst = nc.sync.dma_start(dst, src)
            if _prev_dma[0] is not None:
                add_dep_helper(inst.ins, _prev_dma[0].ins, sync=False, reason="dma order")
            _prev_dma[0] = inst
            return inst

        ones_c = const.tile([128, 1], F32, tag="ones_c")
        nc.vector.memset(ones_c, 1.0)
        # bc matmul stationary = sqrt(E): broadcasts sqrt(E)*rsmu, folding the
        # 1/sqrt(E) scale left in rsmu by the E-scaled variance chain
        sqe_r = const.tile([1, 128], F32, tag="sqe_r")
        nc.vector.memset(sqe_r, float(np.sqrt(E)))
        eps_c = const.tile([1, 1], F32, tag="eps_c")
        nc.vector.memset(eps_c, float(E * EPS))

        tabs = const.tile([128, TABC], F32, tag="tabs")
        # KV bias rows at partition 32*l (rank-1 bias matmul operands)
        bkv_all = const.tile([128, 2 * E], dtw, tag="bkv_all")
        ones32 = const.tile([128, 128], dtw, tag="ones32")
        nc.vector.memset(ones32, 1.0)
        # scratch target for ACT-table preload dummies (written, never read --
        # the walrus no-reader warning is benign). Swapping the ACT LUT costs
        # ~1.3us; preloading via a 1-element op moves that off the LN chain.
        scr = const.tile([1, 4], F32, tag="scr")

        def act_preload(func, gate):
            # `gate` pins the dummy into its phase: without a data dep the
            # scheduler hoists all preloads to kernel start, where they
            # thrash the table uselessly.
            nc.scalar.activation(scr[0:1, 0:1], gate[0:1, 0:1], func)

        def emit_stats(stat, k, x_tile, sq_tile):
            """Accumulate per-token sum / sumsq into stat row 0 via ones-matmuls.

            ONE accumulation group for both column regions: start=True clears
            has_written for the whole bank, so each region's k>0 matmuls
            accumulate (bit set by that region's k==0 overwrite) while the
            other region is untouched. Two interleaved groups would be wrong
            (the second group's start wipes the first group's bits)."""
            nc.tensor.matmul(
                stat[:, 0:T], ones_c, x_tile, start=(k == 0), stop=False,
                skip_group_check=True,
            )
            nc.tensor.matmul(
                stat[:, T : 2 * T], ones_c, sq_tile, start=False,
                stop=(k == KT - 1), skip_group_check=True,
            )

        def emit_stats_all(stat, x_tiles, sq_tiles):
            for k in range(KT):
                nc.tensor.matmul(
                    stat[:, 0:T], ones_c, x_tiles[k], start=(k == 0), stop=(k == KT - 1)
                )
            for k in range(KT):
                nc.tensor.matmul(
                    stat[:, T : 2 * T], ones_c, sq_tiles[k],
                    start=(k == 0), stop=(k == KT - 1),
                )

        def ln_finish(x_tiles, stat, out_dt, out_tag, out_bufs, gcol=None, bcol=None):
            """Finish LN given accumulated stats: (x - mu) * rsqrt(var + eps).

            Works in E-scaled variance space to keep the serial chain short:
            V = sumsq - sum^2/E = E*var; r' = 1/sqrt(V + E*eps) = rs/sqrt(E).
            rsmu holds [r' | sum*r'/E]; the bc broadcast matmul's stationary
            is sqrt(E), so bc = [rs | mu*rs] as the applies expect."""
            s2 = apool.tile([1, T], F32, tag="mu2", bufs=2)
            nc.scalar.activation(s2, stat[:, 0:T], AF.Square)
            var = apool.tile([1, T], F32, tag="var", bufs=2)
            nc.vector.scalar_tensor_tensor(
                var, s2, -1.0 / E, stat[:, T : 2 * T], op0=AO.mult, op1=AO.add
            )
            sd = apool.tile([1, T], F32, tag="sd", bufs=2)
            nc.scalar.activation(sd, var, AF.Sqrt, bias=eps_c)
            rsmu = apool.tile([1, 2 * T], F32, tag="rsmu", bufs=2)
            nc.vector.reciprocal_approx_fast(rsmu[:, 0:T], sd)
            nc.vector.scalar_tensor_tensor(
                rsmu[:, T : 2 * T],
                stat[:, 0:T],
                1.0 / E,
                rsmu[:, 0:T],
                op0=AO.mult,
                op1=AO.mult,
            )
            bc = pstat.tile([128, 512], F32, tag="st", bufs=2, name="bc")
            nc.tensor.matmul(bc, sqe_r, rsmu, start=True, stop=True)
            outs = []
            for k in range(KT):
                tmp = apool.tile([128, T], F32, tag="lntmp", bufs=2)
                ot = apool.tile([128, T], out_dt, tag=out_tag, bufs=out_bufs)
                if gcol is None and k == 0:
                    # half-width split: the first consumer matmul only needs
                    # cols 0:128, so let it start half an apply earlier
                    for hh in range(2):
                        sl = slice(hh * 128, (hh + 1) * 128)
                        sm = slice(T + hh * 128, T + (hh + 1) * 128)
                        nc.vector.tensor_mul(tmp[:, sl], x_tiles[k][:, sl], bc[:, sl])
                        nc.vector.tensor_sub(ot[:, sl], tmp[:, sl], bc[:, sm])
                    outs.append(ot)
                    continue
                nc.vector.tensor_mul(tmp, x_tiles[k], bc[:, 0:T])
                if gcol is None:
                    nc.vector.tensor_sub(ot, tmp, bc[:, T : 2 * T])
                else:
                    tmp2 = apool.tile([128, T], F32, tag="lntmp2", bufs=2)
                    nc.vector.tensor_sub(tmp2, tmp, bc[:, T : 2 * T])
                    nc.vector.tensor_scalar(
                        ot, tmp2, gcol[:, k : k + 1], bcol[:, k : k + 1],
                        op0=AO.mult, op1=AO.add,
                    )
                outs.append(ot)
            return outs

        def load_qkvo(l):
            """Queue the attention-side weight DMAs for layer l."""
            wkv_sb = []
            for k in range(KT):
                t = wpool.tile([128, 2 * E], dtw, tag="wkv", bufs=6)
                sdma(t, wkv_d[l, k * 128 : (k + 1) * 128, :])
                wkv_sb.append(t)
            wq_sb = []
            for k in range(KT):
                t = wpool.tile([128, E], dtw, tag="wq", bufs=7)
                sdma(t, wq_d[l, k * 128 : (k + 1) * 128, :])
                wq_sb.append(t)
            wo_sb = []
            for k in range(KT):
                t = wpool.tile([128, E], dtw, tag="wo", bufs=7)
                sdma(t, wo_d[l, k * 128 : (k + 1) * 128, :])
                wo_sb.append(t)
            return wkv_sb, wq_sb, wo_sb

        def layer(l, xT, stat, w_cur):
            bq_sb = tabs[:, TAB_BQ + l * KT : TAB_BQ + (l + 1) * KT]
            bo_sb = tabs[:, TAB_BO + l * KT : TAB_BO + (l + 1) * KT]
            b2_sb = tabs[:, TAB_B2 + l * KT : TAB_B2 + (l + 1) * KT]
            b1_sb = tabs[:, TAB_B1 + l * FT : TAB_B1 + (l + 1) * FT]
            wkv_sb, wq_sb, wo_sb = w_cur

            # ---- K,V PSUM banks + rank-1 bias matmuls, emitted BEFORE the
            # LN1 chain: they depend only on consts, so the PE chews them
            # while the scalar chain computes rsqrt stats (gap fill).
            kv_ps = [
                [
                    ps.tile([128, 512], F32, tag="pp", bufs=PP, name=f"kv_ps_{m}_{n}")
                    for n in range(3)
                ]
                for m in range(2)
            ]
            for m in range(2):
                for n in range(3):
                    nc.tensor.matmul(
                        kv_ps[m][n],
                        ones32[32 * l : 32 * l + 1, :],
                        bkv_all[32 * l : 32 * l + 1, n * 512 : (n + 1) * 512],
                        start=True,
                        stop=False,
                        tile_position=(32 * l, 0),
                    )

            # ---- LN1 finish ----
            hT = ln_finish(xT, stat, dtw, "hT", 6)

            # ---- K,V: stationary = hT slices, moving = Wkv (N-layout out) ----
            for k in range(KT):
                for m in range(2):
                    for n in range(3):
                        nc.tensor.matmul(
                            kv_ps[m][n],
                            hT[k][:, m * 128 : (m + 1) * 128],
                            wkv_sb[k][:, n * 512 : (n + 1) * 512],
                            start=False,
                            stop=(k == KT - 1),
                        )
            KV = []
            for m in range(2):
                kvt = apool.tile([128, 2 * E], dtw, tag="KV", bufs=2)
                for n in range(3):
                    # split the PSUM->SBUF cast across DVE and ACT so the
                    # serial copy tail gating KtV halves
                    dst = kvt[:, n * 512 : (n + 1) * 512]
                    if n % 2 == 0:
                        nc.vector.tensor_copy(dst, kv_ps[m][n])
                    else:
                        nc.scalar.activation(dst, kv_ps[m][n], AF.Copy)
                KV.append(kvt)

            # ---- K^T V partials (contraction over local tokens) ----
            ktv_ps = ps.tile([128, 512], F32, tag="pp", bufs=PP, name="ktv_ps")[
                :, 0 : 6 * DH
            ]
            for j in range(6):
                for i in range(2):
                    h = 2 * j + i
                    for m in range(2):
                        nc.tensor.matmul(
                            ktv_ps[i * 64 : (i + 1) * 64, j * 64 : (j + 1) * 64],
                            KV[m][:, h * DH : (h + 1) * DH],
                            KV[m][:, E + h * DH : E + (h + 1) * DH],
                            start=(m == 0),
                            stop=(m == 1),
                            tile_position=(0, i * 64),
                        )
            ardt = F32 if AR_F32 else dtw
            ktv_sb = apool.tile([128, 6 * DH], ardt, tag="ktv_sb", bufs=2)
            nc.vector.tensor_copy(ktv_sb[:, 0 : 3 * DH], ktv_ps[:, 0 : 3 * DH])
            nc.scalar.activation(
                ktv_sb[:, 3 * DH : 6 * DH], ktv_ps[:, 3 * DH : 6 * DH], AF.Copy
            )

            if collective:
                cc_in = dram.tile([128, 6 * DH], ardt, tag="cc_in", bufs=2)
                cc_out = dram.tile(
                    [128, 6 * DH], ardt, tag="cc_out", bufs=2, addr_space="Shared"
                )
                nc.scalar.dma_start(cc_in, ktv_sb)
                nc.gpsimd.collective_compute(
                    "AllReduce",
                    AO.add,
                    ins=[cc_in.opt()],
                    outs=[cc_out.opt()],
                    replica_groups=[list(range(N_CORES))],
                )
                ktv_f = apool.tile([128, 6 * DH], ardt, tag="ktv_w", bufs=2)
                # split the result fetch across the scalar + sync queues so the
                # halves land in parallel and the first a-matmuls start early
                ar_out = nc.scalar.dma_start(ktv_f[:, 0 : 3 * DH], cc_out[:, 0 : 3 * DH])
                sdma(ktv_f[:, 3 * DH : 6 * DH], cc_out[:, 3 * DH : 6 * DH])
            else:
                ktv_f = ktv_sb
                ar_out = None
            if ardt == dtw:
                ktv_w = ktv_f
            else:
                ktv_w = apool.tile([128, 6 * DH], dtw, tag="ktv_c", bufs=2)
                nc.vector.tensor_copy(ktv_w, ktv_f)

            # ---- MLP weight stream, gated on the AllReduce output landing:
            # the mesh collective is latency-bound and shares HBM ports +
            # SDMA engines with the weight stream, so keep HBM quiet until
            # the AR result is in SBUF. w1/w2 then stream during a/Wo/LN2,
            # comfortably ahead of the MLP matmuls that consume them.
            w1_sb = []
            first_w = [True]

            def wdma(t, src):
                inst = sdma(t, src)
                if first_w[0] and ar_out is not None:
                    add_dep_helper(
                        inst.ins, ar_out.ins, sync=True, reason="quiet HBM during AR"
                    )
                first_w[0] = False

            for fc in range(4):
                for k in range(KT):
                    t = wpool.tile([128, E], dtw, tag="w1", bufs=24 if not NO_ILV else 22)
                    wdma(t, w1_d[l, k * 128 : (k + 1) * 128, fc * E : (fc + 1) * E])
                    w1_sb.append(t)
            w2_sb = []
            for ki in range(FT):
                t = wpool.tile([128, E], dtw, tag="w2", bufs=24 if not NO_ILV else 22)
                wdma(t, w2_d[l, ki * 128 : (ki + 1) * 128, :])
                w2_sb.append(t)
            # next layer's attention weights ride the same stream; they land
            # during this layer's MLP, so the next AR window is HBM-quiet too
            w_next = load_qkvo(l + 1) if l + 1 < n_layers else None

            # ---- Q^T: stationary = Wq columns, moving = hT (T-layout out) ----
            QT = []
            for m in range(KT):
                qps = ps.tile([128, 512], F32, tag="pp", bufs=PP, name="q_ps")[:, 0:T]
                for k in range(KT):
                    nc.tensor.matmul(
                        qps,
                        wq_sb[k][:, m * 128 : (m + 1) * 128],
                        hT[k],
                        start=(k == 0),
                        stop=(k == KT - 1),
                    )
                qt = apool.tile([128, T], dtw, tag="QT", bufs=6)
                nc.vector.tensor_scalar(
                    qt, qps, bq_sb[:, m : m + 1], None, op0=AO.add
                )
                QT.append(qt)

            # ---- a^T: lhsT = KtV[d1, d2] slice, rhs = Q^T head ----
            a_ps = [
                ps.tile([128, 512], F32, tag="pp", bufs=PP, name=f"a_ps_{j}")[:, 0:T]
                for j in range(6)
            ]
            for j in range(6):
                for i in range(2):
                    nc.tensor.matmul(
                        a_ps[j][i * 64 : (i + 1) * 64, :],
                        ktv_w[i * 64 : (i + 1) * 64, j * 64 : (j + 1) * 64],
                        QT[j][i * 64 : (i + 1) * 64, :],
                        start=True,
                        stop=True,
                        tile_position=(i * 64, i * 64),
                    )
            aT = []
            for j in range(6):
                at = apool.tile([128, T], dtw, tag="aT", bufs=6)
                if j % 2 == 0:
                    nc.scalar.activation(at, a_ps[j], AF.Copy)
                else:
                    nc.vector.tensor_copy(at, a_ps[j])
                aT.append(at)

            # ---- o = a @ Wo + bo + x (residual), LN2 stats interleaved ----
            stat2 = pstat.tile([128, 512], F32, tag="st", bufs=2, name="stat2")[0:1, :]
            x2T = []
            sq2 = []
            for m in range(KT):
                ops_ = ps.tile([128, 512], F32, tag="pp", bufs=PP, name="o_ps")[:, 0:T]
                for k in range(KT):
                    nc.tensor.matmul(
                        ops_,
                        wo_sb[k][:, m * 128 : (m + 1) * 128],
                        aT[k],
                        start=(k == 0),
                        stop=(k == KT - 1),
                    )
                x2 = apool.tile([128, T], F32, tag="x2T", bufs=6)
                nc.vector.scalar_tensor_tensor(
                    x2, ops_, bo_sb[:, m : m + 1], xT[m], op0=AO.add, op1=AO.add
                )
                x2T.append(x2)
                sq = apool.tile([128, T], F32, tag="sq2", bufs=6 if NO_ILV else 3)
                nc.scalar.activation(sq, x2, AF.Square)
                sq2.append(sq)
                if not NO_ILV and m >= 2:
                    emit_stats(stat2, m - 2, x2T[m - 2], sq2[m - 2])
            if NO_ILV:
                emit_stats_all(stat2, x2T, sq2)
            else:
                for m in (KT - 2, KT - 1):
                    emit_stats(stat2, m, x2T[m], sq2[m])

            # ---- LN2 finish ----
            h2T = ln_finish(x2T, stat2, dtw, "hT", 6)
            act_preload(AF.Gelu, h2T[KT - 1])

            # ---- MLP1: z = gelu(h2 @ W1 + b1), all 4 chunks ----
            zc = []
            for fc in range(4):
                z_ps = [
                    ps.tile([128, 512], F32, tag="pp", bufs=PP, name=f"z_ps_{fc}_{f}")[
                        :, 0:T
                    ]
                    for f in range(6)
                ]
                for k in range(KT):
                    for f in range(6):
                        nc.tensor.matmul(
                            z_ps[f],
                            w1_sb[fc * KT + k][:, f * 128 : (f + 1) * 128],
                            h2T[k],
                            start=(k == 0),
                            stop=(k == KT - 1),
                        )
                for f in range(6):
                    zt = apool.tile([128, T], dtw, tag="zT", bufs=24)
                    fi = fc * 6 + f
                    nc.scalar.activation(
                        zt, z_ps[f], AF.Gelu, bias=b1_sb[:, fi : fi + 1]
                    )
                    zc.append(zt)

            act_preload(AF.Sqrt, zc[FT - 1])

            # ---- MLP2 m-outer: next-layer LN1 stats interleaved ----
            stat_n = pstat.tile([128, 512], F32, tag="st", bufs=2, name="stat_n")[0:1, :]
            nxt = []
            sq1 = []
            for m in range(KT):
                mp = ps.tile([128, 512], F32, tag="pp", bufs=PP, name="m_ps")[:, 0:T]
                for ki in range(FT):
                    nc.tensor.matmul(
                        mp,
                        w2_sb[ki][:, m * 128 : (m + 1) * 128],
                        zc[ki],
                        start=(ki == 0),
                        stop=(ki == FT - 1),
                    )
                xn = apool.tile([128, T], F32, tag="xT", bufs=7)
                nc.vector.scalar_tensor_tensor(
                    xn, mp, b2_sb[:, m : m + 1], x2T[m], op0=AO.add, op1=AO.add
                )
                nxt.append(xn)
                sq = apool.tile([128, T], F32, tag="sq1", bufs=6 if NO_ILV else 2)
                nc.scalar.activation(sq, xn, AF.Square)
                sq1.append(sq)
                if not NO_ILV and m >= 1:
                    emit_stats(stat_n, m - 1, nxt[m - 1], sq1[m - 1])
            if NO_ILV:
                emit_stats_all(stat_n, nxt, sq1)
            else:
                emit_stats(stat_n, KT - 1, nxt[KT - 1], sq1[KT - 1])
            return nxt, stat_n, w_next

        for _rep in range(reps):
            # ---- input: load slice, add wpe via DMA inline-add, transpose ----
            # barrier AllReduce: absorbs cross-core start skew / first-
            # collective setup cost concurrently with the input phase, so
            # layer 0's real AllReduce runs as fast as the later ones. The
            # gpsimd trigger blocks that engine until done; nothing else
            # uses gpsimd before layer 0's trigger.
            bz = const.tile([1, DH], F32, tag="bz")
            nc.vector.memset(bz, 0.0)
            bar_in = dram.tile([1, DH], F32, tag="bar_in", bufs=1)
            bar_out = dram.tile([1, DH], F32, tag="bar_out", bufs=1, addr_space="Shared")
            nc.scalar.dma_start(bar_in, bz)
            nc.gpsimd.collective_compute(
                "AllReduce",
                AO.add,
                ins=[bar_in.opt()],
                outs=[bar_out.opt()],
                replica_groups=[list(range(N_CORES))],
            )
            sdma(tabs, tabs_d)
            for _l in range(L):
                sdma(bkv_all[32 * _l : 32 * _l + 1, :], bkv_d[_l].unsqueeze(0))
            stat = pstat.tile([128, 512], F32, tag="st", bufs=2, name="stat_in")[0:1, :]
            xT = []
            sq0 = []
            for k in range(KT):
                xt = apool.tile([128, T], F32, tag="xT", bufs=7)
                sdma(xt, xt_d[k * 128 : (k + 1) * 128, :])
                xT.append(xt)
                sq = apool.tile([128, T], F32, tag="sq1", bufs=6 if NO_ILV else 2)
                nc.scalar.activation(sq, xt, AF.Square)
                sq0.append(sq)
                if not NO_ILV and k >= 1:
                    emit_stats(stat, k - 1, xT[k - 1], sq0[k - 1])
            if NO_ILV:
                emit_stats_all(stat, xT, sq0)
            else:
                emit_stats(stat, KT - 1, xT[KT - 1], sq0[KT - 1])

            act_preload(AF.Sqrt, xT[KT - 1])
            w_cur = load_qkvo(0)
            for l in range(n_layers):
                xT, stat, w_cur = layer(l, xT, stat, w_cur)

            # ---- final LN (with gain/bias) + transpose back + store ----
            fT = ln_finish(
                xT, stat, F32, "fT", 7,
                gcol=tabs[:, TAB_LNFG : TAB_LNFG + KT],
                bcol=tabs[:, TAB_LNFB : TAB_LNFB + KT],
            )
            for k in range(KT):
                sdma(out_d[k * 128 : (k + 1) * 128, :], fT[k])

    nc.compile()
    return nc


class SpmdRunner:
    """Reusable jitted SPMD runner (modeled on bass2jax.run_bass_via_pjrt,
    without donation, so it can be invoked repeatedly)."""

    def __init__(self, nc, n_cores=N_CORES):
        bass2jax.install_neuronx_cc_hook()
        self.nc = nc
        self.n_cores = n_cores
        partition_name = nc.partition_id_tensor.name if nc.partition_id_tensor else None
        in_names, out_names, out_avals = [], [], []
        for alloc in nc.m.functions[0].allocations:
            if not isinstance(alloc, mybir.MemoryLocationSet):
                continue
            name = alloc.memorylocations[0].name
            if alloc.kind == "ExternalInput":
                if name != partition_name:
                    in_names.append(name)
            elif alloc.kind == "ExternalOutput":
                out_names.append(name)
                out_avals.append(
                    jax.core.ShapedArray(
                        tuple(alloc.tensor_shape), mybir.dt.np(alloc.dtype)
                    )
                )
        self.in_names, self.out_names, self.out_avals = in_names, out_names, out_avals
        n_params = len(in_names)
        all_in_names = list(in_names) + list(out_names)
        if partition_name is not None:
            all_in_names.append(partition_name)

        def _body(*args):
            operands = list(args)
            if partition_name is not None:
                operands.append(bass2jax.partition_id_tensor())
            outs = bass2jax._bass_exec_p.bind(
                *operands,
                out_avals=tuple(out_avals),
                in_names=tuple(all_in_names),
                out_names=tuple(out_names),
                lowering_input_output_aliases=(),
                sim_require_finite=True,
                sim_require_nnan=True,
                nc=nc,
            )
            return tuple(outs)

        devices = jax.devices()[:n_cores]
        self.mesh = Mesh(np.asarray(devices), ("core",))
        n_outs = len(out_names)
        in_specs = (PartitionSpec("core"),) * (n_params + n_outs)
        out_specs = (PartitionSpec("core"),) * n_outs
        self.fn = jax.jit(
            shard_map(
                _body,
                mesh=self.mesh,
                in_specs=in_specs,
                out_specs=out_specs,
                check_rep=False,
            ),
            keep_unused=True,
        )
        self.args = None

    def stage(self, in_maps):
        n = self.n_cores
        concat_in = [
            np.concatenate([np.asarray(in_maps[c][name]) for c in range(n)], axis=0)
            for name in self.in_names
        ]
        concat_zero = [
            np.zeros((n * a.shape[0], *a.shape[1:]), a.dtype) for a in self.out_avals
        ]
        sh = NamedSharding(self.mesh, PartitionSpec("core"))
        self.args = [jax.device_put(a, sh) for a in concat_in + concat_zero]

    def run(self):
        return self.fn(*self.args)

    def results(self, out_arrs):
        n = self.n_cores
        return [
            {
                name: np.asarray(out_arrs[i]).reshape(n, *self.out_avals[i].shape)[c]
                for i, name in enumerate(self.out_names)
            }
            for c in range(n)
        ]


def preprocess(inputs):
    """Host-side: fold LN gains into weights, pack bias tables, shard tokens."""
    f = np.float32
    ie = np.asarray(inputs["inputs_embeds"], f)[0]  # [S, E]
    wpe = np.asarray(inputs["wpe"], f)[:S]
    g1 = np.asarray(inputs["ln1_g"], f)
    b1l = np.asarray(inputs["ln1_b"], f)
    g2 = np.asarray(inputs["ln2_g"], f)
    b2l = np.asarray(inputs["ln2_b"], f)
    Wq = np.asarray(inputs["Wq"], f)
    Wk = np.asarray(inputs["Wk"], f)
    Wv = np.asarray(inputs["Wv"], f)
    Wo = np.asarray(inputs["Wo"], f)
    W1 = np.asarray(inputs["W1"], f)
    W2 = np.asarray(inputs["W2"], f)
    bq = np.asarray(inputs["bq"], f)
    bk = np.asarray(inputs["bk"], f)
    bv = np.asarray(inputs["bv"], f)
    bo = np.asarray(inputs["bo"], f)
    b1 = np.asarray(inputs["b1"], f)
    b2 = np.asarray(inputs["b2"], f)

    scale = 1.0 / np.sqrt(DH)
    Wq_p = g1[:, :, None] * Wq * scale
    bq_p = (np.einsum("le,lef->lf", b1l, Wq) + bq) * scale
    Wk_p = g1[:, :, None] * Wk
    bk_p = np.einsum("le,lef->lf", b1l, Wk) + bk
    Wv_p = g1[:, :, None] * Wv
    bv_p = np.einsum("le,lef->lf", b1l, Wv) + bv
    Wkv = np.concatenate([Wk_p, Wv_p], axis=2)
    bkv = np.concatenate([bk_p, bv_p], axis=1)
    W1_p = g2[:, :, None] * W1
    b1_p = np.einsum("le,lef->lf", b2l, W1) + b1

    if COMPUTE == "bf16":
        cast = lambda a: np.ascontiguousarray(a).astype(ml_dtypes.bfloat16)
    else:
        cast = lambda a: np.ascontiguousarray(a, f)

    # packed f32 tables: column (l*KT + o) on partition p = bias[l, o*128 + p]
    def pack(b, nt):
        return b.reshape(L, nt, 128).transpose(2, 0, 1).reshape(128, L * nt)

    tabs = np.concatenate(
        [
            pack(bq_p, KT), pack(np.ascontiguousarray(bo), KT),
            pack(np.ascontiguousarray(b2), KT), pack(b1_p, FT),
            np.asarray(inputs["lnf_g"], f).reshape(KT, 128).T,
            np.asarray(inputs["lnf_b"], f).reshape(KT, 128).T,
        ],
        axis=1,
    )


    common = {
        "wq": cast(Wq_p),
        "wkv": cast(Wkv),
        "wo": cast(Wo),
        "w1": cast(W1_p),
        "w2": cast(np.ascontiguousarray(W2)),
        "tabs": np.ascontiguousarray(tabs, f),
        "bkv": cast(bkv),
    }
    x0 = ie + wpe  # [S, E]
    maps = []
    for c in range(N_CORES):
        sl = slice(c * T, (c + 1) * T)
        maps.append(
            {
                **common,
                "xt": np.ascontiguousarray(x0[sl].T),
            }
        )
    return maps


_RUNNER = None


def _get_runner():
    global _RUNNER
    if _RUNNER is None:
        nc = build_model(reps=1)
        _RUNNER = SpmdRunner(nc)
    return _RUNNER


def kernel(**inputs):
    runner = _get_runner()
    maps = preprocess(inputs)
    runner.stage(maps)
    outs = runner.run()
    res = runner.results(outs)
    full = np.concatenate([res[c]["out"].T for c in range(N_CORES)], axis=0)
    return full[None].astype(np.float32)
